# revision 33
# baseline (speedup 1.0000x reference)
"""GCN classifier with metrics — TRN2 Bass kernel (8 NeuronCores, SPMD).

v2 strategy:
  - Nodes partitioned contiguously across 8 cores (12500/core). Features kept
    64-wide; table rows PACK TWO NODES per 128-col bf16 row (256B), halving
    the AllGather vs a padded table. Node (core c, local v) lives at table
    row c*6272 + (v % 6272), half v // 6272.
  - Per layer: h_scaled = h * dinv computed per-shard into an SBUF store,
    written packed to DRAM, AllGathered into table [50176, 128] bf16.
  - Edge aggregation per 128-dst tile: slots binned by (tile, half, window);
    dma_gather (int16 window-relative idx over 2 windows of 25088 rows)
    fetches slot rows; one-hot S generated ON-CHIP via DVE is_equal from a
    streamed per-slot dst-partition id (f32, -1 for padding); PE matmul
    accumulates agg[d,:] += sum_s S[s,d]*msg[s, half*64:half*64+64] in PSUM.
  - Self loops NEVER enter the edge path: added in the epilogue from the
    SBUF h_scaled store (agg_total = (agg + h_sc[v]) * dinv[v]).
  - global_mean_pool: indicator matmul per tile into PSUM over two 128-graph
    windows, AllReduce [256,17], head computed redundantly per core.
"""
import sys
import numpy as np

sys.path.insert(0, "/opt/trn_rl_repo")

import ml_dtypes
import concourse.bass as bass
import concourse.bacc as bacc
import concourse.mybir as mybir
import concourse.tile as tile
from concourse.bass_utils import run_bass_kernel_spmd
from concourse.library_config import mlp as mlp_lib

BF16 = ml_dtypes.bfloat16

N = 100_000
E = 1_600_000
G = 256
CIN = 128
NCLS = 10
NCORES = 8
SHARD = 12_500
NT = 98                     # dst tiles per core (98*128 = 12544)
HROWS = 6_272               # packed table rows per core (2 nodes/row)
TROWS = NCORES * HROWS      # 50176 table rows
WIN = 25_088                # rows per idx window (int16-safe)
NWIN = 2
H1 = 64
H2 = 16
GROUP_T = 4                 # dst tiles per group
MAX_CALL_BLK = 8            # blocks per dma_gather call (<=1024 idx)
F32 = mybir.dt.float32
BF = mybir.dt.bfloat16
I16 = mybir.dt.int16


def _wrap_idx(idx):
    """[n] int16 (n % 128 == 0) -> [128, n//16] wrapped + replicated layout."""
    n = len(idx)
    w = idx.reshape(n // 16, 16).T.astype(np.int16)   # [16, n/16]
    return np.tile(w, (8, 1))


def _build_structure(src, dst):
    """Shared (SPMD-uniform) block/call schedule + per-core idx/pp data.

    Returns (schedule, per_core):
      schedule: groups -> calls (window, col/blk offsets, per-block (tile,half))
      per_core: list of dicts with idx [128, totcol] i16, pp [128, totblk] f32
    """
    # node -> packed table coordinates
    nodes = np.arange(N)
    ncore = nodes // SHARD
    nloc = nodes % SHARD
    row_g = ncore * HROWS + (nloc % HROWS)
    half_g = nloc // HROWS                       # 0 or 1
    win_g = row_g // WIN
    rel_g = (row_g - win_g * WIN).astype(np.int32)

    ecore = dst // SHARD
    order = np.argsort(ecore, kind="stable")
    src_o, dst_o = src[order], dst[order]
    core_bounds = np.searchsorted(ecore[order], np.arange(NCORES + 1))

    NBIN = NT * 2 * NWIN                          # (t, h, w)
    counts = np.zeros((NCORES, NBIN), np.int64)
    per_core_slots = []
    for c in range(NCORES):
        s_c = src_o[core_bounds[c]:core_bounds[c + 1]]
        d_c = dst_o[core_bounds[c]:core_bounds[c + 1]]
        dloc = d_c - c * SHARD
        t_all = dloc // 128
        p_all = dloc % 128
        h_all = half_g[s_c]
        w_all = win_g[s_c]
        rel = rel_g[s_c]
        binid = (t_all * 2 + h_all) * NWIN + w_all
        o2 = np.argsort(binid * 32768 + rel, kind="stable")
        binid, rel, p_all = binid[o2], rel[o2], p_all[o2]
        cnt = np.bincount(binid, minlength=NBIN)
        counts[c] = cnt
        bounds = np.concatenate([[0], np.cumsum(cnt)])
        per_core_slots.append((rel.astype(np.int32), p_all.astype(np.int32),
                               bounds))

    maxc = counts.max(axis=0).reshape(NT, 2, NWIN)
    nblk = (maxc + 127) // 128                   # blocks per (t, h, w)

    groups = []
    blk_off = 0
    col_off = 0
    for g0 in range(0, NT, GROUP_T):
        tiles = list(range(g0, min(NT, g0 + GROUP_T)))
        calls = []
        for w in range(NWIN):
            wblocks = []                          # [(t, h, blk_within)]
            for t in tiles:
                for h in range(2):
                    for b in range(int(nblk[t, h, w])):
                        wblocks.append((t, h, b))
            i = 0
            while i < len(wblocks):
                chunk = wblocks[i:i + MAX_CALL_BLK]
                calls.append({
                    "w": w,
                    "nb": len(chunk),
                    "col": col_off,
                    "blk": blk_off,
                    "blocks": chunk,             # (tile, half, b_within)
                })
                blk_off += len(chunk)
                col_off += len(chunk) * 8
                i += MAX_CALL_BLK
        groups.append({"tiles": tiles, "calls": calls})
    totblk, totcol = blk_off, col_off

    per_core = []
    for c in range(NCORES):
        rel, part, bounds = per_core_slots[c]
        idx_cols = np.zeros((128, totcol), np.int16)
        pp = np.full((128, totblk), -1.0, BF16)
        for g in groups:
            for call in g["calls"]:
                w = call["w"]
                stream = np.zeros(call["nb"] * 128, np.int16)
                for j, (t, h, b) in enumerate(call["blocks"]):
                    bid = (t * 2 + h) * NWIN + w
                    lo, hi = bounds[bid], bounds[bid + 1]
                    s0 = lo + b * 128
                    s1 = min(hi, s0 + 128)
                    nsl = s1 - s0
                    if nsl > 0:
                        stream[j * 128:j * 128 + nsl] = rel[s0:s1]
                        pp[:nsl, call["blk"] + j] = part[s0:s1]
                idx_cols[:, call["col"]:call["col"] + call["nb"] * 8] = \
                    _wrap_idx(stream)
        per_core.append({"idx": idx_cols, "pp": pp})

    sched = {"groups": groups, "totblk": totblk, "totcol": totcol}
    return sched, per_core


def _build_program(sched):
    nc = bacc.Bacc("TRN2", target_bir_lowering=False, debug=False,
                   num_devices=NCORES, num_swdge_queues=4)
    totblk, totcol = sched["totblk"], sched["totcol"]

    def inp(name, shape, dt=F32):
        return nc.declare_dram_parameter(name, shape, dt, isOutput=False)

    xT = inp("xT", [CIN, NT * 128])
    dinv = inp("dinv", [128, NT])
    batchf = inp("batchf", [128, NT])
    iota01 = inp("iota01", [128, 256])
    iota128 = inp("iota128", [128, 128], BF)
    ident = inp("ident", [128, 128])
    idxT = inp("idx", [128, totcol], I16)
    ppT = inp("pp", [128, totblk], BF)
    W1 = inp("W1", [CIN, H1]);  Wr1 = inp("Wr1", [CIN, H1])
    W2 = inp("W2", [H1, H2]);   Wr2 = inp("Wr2", [H1, H2])
    b1b = inp("b1b", [128, H1]); br1b = inp("br1b", [128, H1])
    b2b = inp("b2b", [128, H2]); br2b = inp("br2b", [128, H2])
    Wf1t = inp("Wf1t", [16, 80])
    Wf2 = inp("Wf2", [80, NCLS])
    bf2r = inp("bf2r", [1, NCLS])
    mcin = inp("mcin", [1, 80])
    alpha = inp("alpha", [128, 2])
    out = nc.declare_dram_parameter("out", [G, NCLS], F32, isOutput=True)

    SILU = mybir.ActivationFunctionType.Silu
    EQ = mybir.AluOpType.is_equal

    with tile.TileContext(nc) as tc:
        with tc.tile_pool(name="const", bufs=1) as constp, \
             tc.tile_pool(name="store", bufs=1) as storep, \
             tc.tile_pool(name="xp", bufs=4) as xp, \
             tc.tile_pool(name="msg", bufs=20) as msgp, \
             tc.tile_pool(name="stl", bufs=20) as stlp, \
             tc.tile_pool(name="idxp", bufs=6) as idxp, \
             tc.tile_pool(name="ppp", bufs=6) as ppp, \
             tc.tile_pool(name="ep", bufs=6) as ep, \
             tc.tile_pool(name="dram", bufs=1, space="DRAM") as dram, \
             tc.tile_pool(name="ps_tp", bufs=1, space="PSUM") as ps_tp, \
             tc.tile_pool(name="ps_mm", bufs=2, space="PSUM") as ps_mm, \
             tc.tile_pool(name="ps_agg", bufs=4, space="PSUM") as ps_agg, \
             tc.tile_pool(name="ps_pool", bufs=1, space="PSUM") as ps_pool:

            nc.gpsimd.load_library(mlp_lib)

            def ld(ap_src, shape, dt=F32, tag=None):
                t = constp.tile(shape, dt, tag=tag or ap_src.tensor.name)
                nc.sync.dma_start(out=t[:], in_=ap_src)
                return t

            dinv_sb = ld(dinv[:], [128, NT])
            batch_sb = ld(batchf[:], [128, NT])
            iota_sb = ld(iota01[:], [128, 256])
            io128_sb = ld(iota128[:], [128, 128], BF)
            ident_sb = ld(ident[:], [128, 128])
            W1_sb = ld(W1[:], [CIN, H1]); Wr1_sb = ld(Wr1[:], [CIN, H1])
            W2_sb = ld(W2[:], [H1, H2]); Wr2_sb = ld(Wr2[:], [H1, H2])
            b1_sb = ld(b1b[:], [128, H1]); br1_sb = ld(br1b[:], [128, H1])
            b2_sb = ld(b2b[:], [128, H2]); br2_sb = ld(br2b[:], [128, H2])
            Wf1t_sb = ld(Wf1t[:], [16, 80])
            Wf2_sb = ld(Wf2[:], [80, NCLS])
            bf2_sb = ld(bf2r[:], [1, NCLS])
            al_sb = ld(alpha[:], [128, 2])
            ones1 = constp.tile([1, 128], F32, tag="ones1")
            nc.vector.memset(ones1[:], 1.0)

            r1_store = storep.tile([128, NT * H1], F32, tag="r1s")
            r2_store = storep.tile([128, NT * H2], F32, tag="r2s")
            h1sc_store = storep.tile([128, NT * H1], BF, tag="h1sc")
            h2sc_store = storep.tile([128, NT * H1], BF, tag="h2sc")

            h1s_shard = dram.tile([HROWS, 128], BF)
            table1 = dram.tile([TROWS, 128], BF)
            h2s_shard = dram.tile([HROWS, 128], BF)
            table2 = dram.tile([TROWS, 128], BF)
            pool_in = dram.tile([G, 17], F32)
            pool_out = dram.tile([G, 17], F32)

            HB = 49 * H1

            def write_shard(store, shard):
                # shard row r: low half = node r, high half = node 6272+r
                nc.scalar.dma_start(
                    out=shard[:, 0:64].rearrange("(t p) c -> p t c", p=128),
                    in_=store[:, :HB])
                nc.scalar.dma_start(
                    out=shard[:, 64:128].rearrange("(t p) c -> p t c", p=128),
                    in_=store[:, HB:2 * HB])

            def ag(shard, table_):
                nc.gpsimd.collective_compute(
                    "AllGather", mybir.AluOpType.bypass,
                    replica_groups=[list(range(NCORES))],
                    ins=[shard.opt()], outs=[table_.opt()])

            # ---------------- stage 0: h1sc + r1 ----------------
            XB = 4

            def stage0_run(tlist):
                i = 0
                while i < len(tlist):
                    t0 = tlist[i]
                    nt = 1
                    while (nt < XB and i + nt < len(tlist)
                           and tlist[i + nt] == t0 + nt):
                        nt += 1
                    xw = xp.tile([128, XB * 128], F32, tag="xw")
                    nc.scalar.dma_start(
                        out=xw[:, :nt * 128],
                        in_=xT[:, t0 * 128:(t0 + nt) * 128])
                    for a in range(nt):
                        t = t0 + a
                        xTt = xw[:, a * 128:(a + 1) * 128]
                        hpre = ps_mm.tile([128, H1], F32, tag="mm")
                        nc.tensor.matmul(out=hpre[:], lhsT=xTt, rhs=W1_sb[:],
                                         start=True, stop=True)
                        nc.vector.tensor_scalar_mul(
                            out=h1sc_store[:, t * H1:(t + 1) * H1],
                            in0=hpre[:], scalar1=dinv_sb[:, t:t + 1])

                        r1ps = ps_mm.tile([128, H1], F32, tag="mm")
                        nc.tensor.matmul(out=r1ps[:], lhsT=xTt, rhs=Wr1_sb[:],
                                         start=True, stop=True)
                        r1a = ep.tile([128, H1], F32, tag="r1a")
                        nc.vector.tensor_add(out=r1a[:], in0=r1ps[:],
                                             in1=br1_sb[:])
                        nc.scalar.activation(out=r1a[:], in_=r1a[:], func=SILU)
                        nc.vector.tensor_scalar_mul(
                            out=r1_store[:, t * H1:(t + 1) * H1], in0=r1a[:],
                            scalar1=al_sb[:, 0:1])
                    i += nt

            stage0_run(list(range(NT)))
            write_shard(h1sc_store, h1s_shard)
            ag(h1s_shard, table1)

            qctr = [0]

            def run_groups(table_ab, epilogue, hook=None):
                for gi, g in enumerate(sched["groups"]):
                    tiles = g["tiles"]
                    calls = g["calls"]
                    if not calls:
                        for pos, t in enumerate(tiles):
                            epilogue(t, None, pos, len(tiles))
                        continue
                    gcol0 = calls[0]["col"]
                    gblk0 = calls[0]["blk"]
                    gcols = calls[-1]["col"] + calls[-1]["nb"] * 8 - gcol0
                    gblks = calls[-1]["blk"] + calls[-1]["nb"] - gblk0
                    it = idxp.tile([128, gcols], I16, tag="it",
                                   name=f"it{tiles[0]}")
                    nc.scalar.dma_start(out=it[:],
                                        in_=idxT[:, gcol0:gcol0 + gcols])
                    pt = ppp.tile([128, gblks], BF, tag="pt",
                                  name=f"pt{tiles[0]}")
                    nc.scalar.dma_start(out=pt[:],
                                        in_=ppT[:, gblk0:gblk0 + gblks])

                    agg_g = ps_agg.tile([128, GROUP_T * H1], F32, tag="agg",
                                        name=f"agg{tiles[0]}")
                    nc.vector.memset(agg_g[:], 0.0)
                    aggs = {}
                    nb_left = {}
                    for pos, t in enumerate(tiles):
                        aggs[t] = agg_g[:, pos * H1:(pos + 1) * H1]
                        nb_left[t] = sum(
                            1 for call in calls for (tt, _, _) in call["blocks"]
                            if tt == t)

                    for call in calls:
                        nb = call["nb"]
                        mt = msgp.tile([128, MAX_CALL_BLK * 128], BF, tag="mt")
                        st = stlp.tile([128, MAX_CALL_BLK * 128], BF, tag="st")
                        cb0 = call["blk"] - gblk0
                        nc.vector.tensor_tensor(
                            out=st[:, :nb * 128].rearrange(
                                "p (b d) -> p b d", d=128),
                            in0=pt[:, cb0:cb0 + nb].rearrange(
                                "p b -> p b ()").to_broadcast([128, nb, 128]),
                            in1=io128_sb[:].rearrange(
                                "p d -> p () d").to_broadcast([128, nb, 128]),
                            op=EQ)
                        w = call["w"]
                        nc.gpsimd.dma_gather(
                            mt[:, :nb * 128].rearrange("p (b d) -> p b d",
                                                       d=128),
                            table_ab[w],
                            it[:, call["col"] - gcol0:
                               call["col"] - gcol0 + nb * 8],
                            nb * 128, nb * 128, 128,
                            queue_num=qctr[0] % 4,
                        )
                        qctr[0] += 1
                        for j, (t, h, _) in enumerate(call["blocks"]):
                            nb_left[t] -= 1
                            nc.tensor.matmul(
                                out=aggs[t],
                                lhsT=st[:, j * 128:(j + 1) * 128],
                                rhs=mt[:, j * 128 + h * H1:
                                       j * 128 + h * H1 + H1],
                                start=False, stop=(nb_left[t] == 0),
                                skip_group_check=True)
                    ntl = len(tiles)
                    aggsb = ep.tile([128, GROUP_T * H1], F32, tag="aggsb",
                                    name=f"aggsb{tiles[0]}")
                    nc.vector.tensor_copy(out=aggsb[:, :ntl * H1],
                                          in_=agg_g[:, :ntl * H1])
                    for pos, t in enumerate(tiles):
                        epilogue(t, aggsb[:, pos * H1:(pos + 1) * H1], pos,
                                 ntl)
                    if hook is not None:
                        hook(gi)

            # ---------------- stage 1 ----------------
            def epi1(t, agg, pos, ntl):
                asum = ep.tile([128, H1], F32, tag="e1s")
                nc.vector.tensor_add(out=asum[:], in0=agg,
                                     in1=h1sc_store[:, t * H1:(t + 1) * H1])
                a = ep.tile([128, H1], F32, tag="e1a")
                nc.vector.tensor_scalar_mul(out=a[:], in0=asum[:],
                                            scalar1=dinv_sb[:, t:t + 1])
                nc.vector.tensor_add(out=a[:], in0=a[:], in1=b1_sb[:])
                nc.scalar.activation(out=a[:], in_=a[:], func=SILU)
                h = ep.tile([128, H1], F32, tag="e1h")
                nc.vector.tensor_add(out=h[:], in0=a[:],
                                     in1=r1_store[:, t * H1:(t + 1) * H1])
                nc.vector.tensor_scalar_mul(
                    out=h2sc_store[:, t * H1:(t + 1) * H1], in0=h[:],
                    scalar1=dinv_sb[:, t:t + 1])
                hT_ps = ps_tp.tile([128, 128], F32, tag="tp")
                nc.tensor.transpose(out=hT_ps[:H1, :], in_=h[:],
                                    identity=ident_sb[:])
                hT = ep.tile([H1, 128], F32, tag="e1ht")
                nc.vector.tensor_copy(out=hT[:], in_=hT_ps[:H1, :])
                r2ps = ps_mm.tile([128, H2], F32, tag="mm")
                nc.tensor.matmul(out=r2ps[:], lhsT=hT[:], rhs=Wr2_sb[:],
                                 start=True, stop=True)
                r2a = ep.tile([128, H2], F32, tag="e1r2")
                nc.vector.tensor_add(out=r2a[:], in0=r2ps[:], in1=br2_sb[:])
                nc.scalar.activation(out=r2a[:], in_=r2a[:], func=SILU)
                nc.vector.tensor_scalar_mul(
                    out=r2_store[:, t * H2:(t + 1) * H2], in0=r2a[:],
                    scalar1=al_sb[:, 1:2])

            run_groups((table1[0:WIN, :], table1[WIN:2 * WIN, :]), epi1)

            write_shard(h2sc_store, h2s_shard)
            ag(h2s_shard, table2)

            # ---------------- stage 2 + pooling ----------------
            pool_ps = ps_pool.tile([128, 34], F32, tag="pool")
            nc.vector.memset(pool_ps[:], 0.0)
            tcount = [0]

            def epi2(t, agg, pos, ntl):
                asum = ep.tile([128, H1], F32, tag="e2s")
                nc.vector.tensor_add(out=asum[:], in0=agg,
                                     in1=h2sc_store[:, t * H1:(t + 1) * H1])
                a = ep.tile([128, H1], F32, tag="e2a")
                nc.vector.tensor_scalar_mul(out=a[:], in0=asum[:],
                                            scalar1=dinv_sb[:, t:t + 1])
                aT_ps = ps_tp.tile([128, 128], F32, tag="tp")
                nc.tensor.transpose(out=aT_ps[:H1, :], in_=a[:],
                                    identity=ident_sb[:])
                aT = ep.tile([H1, 128], F32, tag="e2at")
                nc.vector.tensor_copy(out=aT[:], in_=aT_ps[:H1, :])
                zps = ps_mm.tile([128, H2], F32, tag="mm")
                nc.tensor.matmul(out=zps[:], lhsT=aT[:], rhs=W2_sb[:],
                                 start=True, stop=True)
                zext = ep.tile([128, H2 + 1], F32, tag="e2z")
                nc.vector.tensor_add(out=zext[:, :H2], in0=zps[:], in1=b2_sb[:])
                nc.vector.tensor_add(out=zext[:, :H2], in0=zext[:, :H2],
                                     in1=r2_store[:, t * H2:(t + 1) * H2])
                nc.vector.memset(zext[:, H2:], 1.0)
                s0 = ep.tile([128, 128], F32, tag="e2s0")
                nc.vector.tensor_tensor(
                    out=s0[:], in0=batch_sb[:, t:t + 1].to_broadcast([128, 128]),
                    in1=iota_sb[:, 0:128], op=EQ)
                k = tcount[0]
                nc.tensor.matmul(out=pool_ps[:, 0:17], lhsT=s0[:], rhs=zext[:],
                                 start=False, stop=(k == NT - 1),
                                 skip_group_check=True)
                s1 = ep.tile([128, 128], F32, tag="e2s1")
                nc.vector.tensor_tensor(
                    out=s1[:], in0=batch_sb[:, t:t + 1].to_broadcast([128, 128]),
                    in1=iota_sb[:, 128:256], op=EQ)
                nc.tensor.matmul(out=pool_ps[:, 17:34], lhsT=s1[:], rhs=zext[:],
                                 start=False, stop=(k == NT - 1),
                                 skip_group_check=True)
                tcount[0] += 1

            run_groups((table2[0:WIN, :], table2[WIN:2 * WIN, :]), epi2)

            psums = ep.tile([128, 34], F32, tag="psums")
            nc.vector.tensor_copy(out=psums[:], in_=pool_ps[:])
            nc.sync.dma_start(out=pool_in[0:128, :], in_=psums[:, 0:17])
            nc.sync.dma_start(out=pool_in[128:256, :], in_=psums[:, 17:34])

            nc.gpsimd.collective_compute(
                "AllReduce", mybir.AluOpType.add,
                replica_groups=[list(range(NCORES))],
                ins=[pool_in.opt()], outs=[pool_out.opt()])

            mc = ep.tile([1, 80], F32, tag="mmc")
            nc.sync.dma_start(out=mc[:], in_=mcin[:])

            # ---------------- classifier head (two graph windows) ----------
            for wdw in range(2):
                sums = ep.tile([128, 17], F32, tag="hsum")
                nc.sync.dma_start(out=sums[:],
                                  in_=pool_out[wdw * 128:(wdw + 1) * 128, :])
                cnt = ep.tile([128, 1], F32, tag="hcnt")
                nc.vector.tensor_scalar_max(out=cnt[:], in0=sums[:, 16:17],
                                            scalar1=1.0)
                rec = ep.tile([128, 1], F32, tag="hrec")
                nc.vector.reciprocal(out=rec[:], in_=cnt[:])
                ge = ep.tile([128, 16], F32, tag="hge")
                nc.vector.tensor_scalar_mul(out=ge[:], in0=sums[:, :16],
                                            scalar1=rec[:])
                geT_ps = ps_tp.tile([128, 128], F32, tag="tp")
                nc.tensor.transpose(out=geT_ps[:16, :], in_=ge[:],
                                    identity=ident_sb[:])
                geT = ep.tile([16, 128], F32, tag="hget")
                nc.vector.tensor_copy(out=geT[:], in_=geT_ps[:16, :])
                u_ps = ps_mm.tile([128, 80], F32, tag="mm")
                nc.tensor.matmul(out=u_ps[:], lhsT=geT[:], rhs=Wf1t_sb[:],
                                 start=True, stop=False)
                nc.tensor.matmul(out=u_ps[:], lhsT=ones1[:], rhs=mc[:],
                                 start=False, stop=True)
                u = ep.tile([128, 80], F32, tag="hu")
                nc.scalar.activation(out=u[:], in_=u_ps[:], func=SILU)
                uT_ps = ps_tp.tile([128, 128], F32, tag="tp")
                nc.tensor.transpose(out=uT_ps[:80, :], in_=u[:],
                                    identity=ident_sb[:])
                uT = ep.tile([80, 128], F32, tag="hut")
                nc.vector.tensor_copy(out=uT[:], in_=uT_ps[:80, :])
                o_ps = ps_mm.tile([128, NCLS], F32, tag="mm")
                nc.tensor.matmul(out=o_ps[:], lhsT=uT[:], rhs=Wf2_sb[:],
                                 start=True, stop=False)
                nc.tensor.matmul(out=o_ps[:], lhsT=ones1[:], rhs=bf2_sb[:],
                                 start=False, stop=True)
                o = ep.tile([128, NCLS], F32, tag="ho")
                nc.vector.tensor_copy(out=o[:], in_=o_ps[:])
                nc.sync.dma_start(out=out[wdw * 128:(wdw + 1) * 128, :],
                                  in_=o[:])

    nc.compile()
    return nc


def _host_metrics_contrib(tolerance, cost, time, quantity,
                          mW1, mb1, mW2, mb2, Wf1, bf1):
    silu = lambda v: v / (1.0 + np.exp(-v))
    m = np.stack([np.asarray(v, np.float32).reshape(1, 1) for v in
                  (tolerance, cost, time, quantity)])         # [4,1,1]
    e = silu(np.einsum('gij,gjk->gik', m, np.asarray(mW1, np.float32))
             + np.asarray(mb1, np.float32)[:, None, :])
    e = (np.einsum('gij,gjk->gik', e, np.asarray(mW2, np.float32))
         + np.asarray(mb2, np.float32)[:, None, :])           # [4,1,16]
    metvec = e.transpose(1, 0, 2).reshape(1, 64)
    mc = (metvec @ np.asarray(Wf1, np.float32)[16:, :]
          + np.asarray(bf1, np.float32)[None, :])
    return mc.astype(np.float32)


def kernel(x, edge_index, batch, tolerance, cost, time, quantity,
           W1, b1, W2, b2, Wr1, br1, Wr2, br2, alpha1, alpha2,
           mW1, mb1, mW2, mb2, Wf1, bf1, Wf2, bf2):
    x = np.asarray(x, np.float32)
    src = np.asarray(edge_index[0], np.int64).astype(np.int64)
    dst = np.asarray(edge_index[1], np.int64).astype(np.int64)
    batch = np.asarray(batch, np.int64)

    deg = 1.0 + np.bincount(dst, minlength=N).astype(np.float32)
    dinv_full = 1.0 / np.sqrt(deg)

    sched, per_core = _build_structure(src, dst)
    nc = _build_program(sched)

    iota01 = np.tile(np.arange(256, dtype=np.float32), (128, 1))
    iota128 = np.tile(np.arange(128, dtype=BF16), (128, 1))
    ident = np.eye(128, dtype=np.float32)
    common = {
        "iota01": iota01, "iota128": iota128, "ident": ident,
        "W1": np.asarray(W1, np.float32), "Wr1": np.asarray(Wr1, np.float32),
        "W2": np.asarray(W2, np.float32), "Wr2": np.asarray(Wr2, np.float32),
        "b1b": np.tile(np.asarray(b1, np.float32), (128, 1)),
        "br1b": np.tile(np.asarray(br1, np.float32), (128, 1)),
        "b2b": np.tile(np.asarray(b2, np.float32), (128, 1)),
        "br2b": np.tile(np.asarray(br2, np.float32), (128, 1)),
        "Wf1t": np.asarray(Wf1[:16, :], np.float32),
        "Wf2": np.asarray(Wf2, np.float32),
        "bf2r": np.asarray(bf2, np.float32)[None, :],
        "mcin": _host_metrics_contrib(tolerance, cost, time, quantity,
                                      mW1, mb1, mW2, mb2, Wf1, bf1),
        "alpha": np.tile(np.array([[float(alpha1), float(alpha2)]],
                                  np.float32), (128, 1)),
    }

    in_maps = []
    for c in range(NCORES):
        lo, hi = c * SHARD, (c + 1) * SHARD
        xs = np.zeros((NT * 128, CIN), np.float32)
        xs[:SHARD] = x[lo:hi]
        dv = np.zeros(NT * 128, np.float32)
        dv[:SHARD] = dinv_full[lo:hi]
        bf_loc = np.full(NT * 128, -1.0, np.float32)
        bf_loc[:SHARD] = batch[lo:hi].astype(np.float32)
        m = dict(common)
        m["xT"] = np.ascontiguousarray(xs.T)
        m["dinv"] = dv.reshape(NT, 128).T.copy()
        m["batchf"] = bf_loc.reshape(NT, 128).T.copy()
        m["idx"] = per_core[c]["idx"]
        m["pp"] = per_core[c]["pp"]
        in_maps.append(m)

    res = run_bass_kernel_spmd(nc, in_maps, list(range(NCORES)))
    kernel._last = (nc, in_maps)   # for external profiling harnesses
    kernel._res = res
    return np.asarray(res.results[0]["out"], np.float32)


# revision 40
# speedup vs baseline: 1.0321x; 1.0321x over previous
"""GCN classifier with metrics — TRN2 Bass kernel (8 NeuronCores, SPMD).

v2 strategy:
  - Nodes partitioned contiguously across 8 cores (12500/core). Features kept
    64-wide; table rows PACK TWO NODES per 128-col bf16 row (256B), halving
    the AllGather vs a padded table. Node (core c, local v) lives at table
    row c*6272 + (v % 6272), half v // 6272.
  - Per layer: h_scaled = h * dinv computed per-shard into an SBUF store,
    written packed to DRAM, AllGathered into table [50176, 128] bf16.
  - Edge aggregation per 128-dst tile: slots binned by (tile, half, window);
    dma_gather (int16 window-relative idx over 2 windows of 25088 rows)
    fetches slot rows; one-hot S generated ON-CHIP via DVE is_equal from a
    streamed per-slot dst-partition id (f32, -1 for padding); PE matmul
    accumulates agg[d,:] += sum_s S[s,d]*msg[s, half*64:half*64+64] in PSUM.
  - Self loops NEVER enter the edge path: added in the epilogue from the
    SBUF h_scaled store (agg_total = (agg + h_sc[v]) * dinv[v]).
  - global_mean_pool: indicator matmul per tile into PSUM over two 128-graph
    windows, AllReduce [256,17], head computed redundantly per core.
"""
import sys
import numpy as np

sys.path.insert(0, "/opt/trn_rl_repo")

import ml_dtypes
import concourse.bass as bass
import concourse.bacc as bacc
import concourse.mybir as mybir
import concourse.tile as tile
from concourse.bass_utils import run_bass_kernel_spmd
from concourse.library_config import mlp as mlp_lib

BF16 = ml_dtypes.bfloat16

N = 100_000
E = 1_600_000
G = 256
CIN = 128
NCLS = 10
NCORES = 8
SHARD = 12_500
NT = 98                     # dst tiles per core (98*128 = 12544)
HROWS = 6_272               # packed table rows per core (2 nodes/row)
TROWS = NCORES * HROWS      # 50176 table rows
WIN = 25_088                # rows per idx window (int16-safe)
NWIN = 2
H1 = 64
H2 = 16
GROUP_T = 4                 # dst tiles per group
MAX_CALL_BLK = 8            # blocks per dma_gather call (<=1024 idx)
F32 = mybir.dt.float32
BF = mybir.dt.bfloat16
I16 = mybir.dt.int16


def _wrap_idx(idx):
    """[n] int16 (n % 128 == 0) -> [128, n//16] wrapped + replicated layout."""
    n = len(idx)
    w = idx.reshape(n // 16, 16).T.astype(np.int16)   # [16, n/16]
    return np.tile(w, (8, 1))


def _build_structure(src, dst):
    """Shared (SPMD-uniform) block/call schedule + per-core idx/pp data.

    Returns (schedule, per_core):
      schedule: groups -> calls (window, col/blk offsets, per-block (tile,half))
      per_core: list of dicts with idx [128, totcol] i16, pp [128, totblk] f32
    """
    # node -> packed table coordinates
    nodes = np.arange(N)
    ncore = nodes // SHARD
    nloc = nodes % SHARD
    row_g = ncore * HROWS + (nloc % HROWS)
    half_g = nloc // HROWS                       # 0 or 1
    win_g = row_g // WIN
    rel_g = (row_g - win_g * WIN).astype(np.int32)

    ecore = dst // SHARD
    order = np.argsort(ecore, kind="stable")
    src_o, dst_o = src[order], dst[order]
    core_bounds = np.searchsorted(ecore[order], np.arange(NCORES + 1))

    NBIN = NT * 2 * NWIN                          # (t, h, w)
    counts = np.zeros((NCORES, NBIN), np.int64)
    per_core_slots = []
    for c in range(NCORES):
        s_c = src_o[core_bounds[c]:core_bounds[c + 1]]
        d_c = dst_o[core_bounds[c]:core_bounds[c + 1]]
        dloc = d_c - c * SHARD
        t_all = dloc // 128
        p_all = dloc % 128
        h_all = half_g[s_c]
        w_all = win_g[s_c]
        rel = rel_g[s_c]
        binid = (t_all * 2 + h_all) * NWIN + w_all
        o2 = np.argsort(binid * 32768 + rel, kind="stable")
        binid, rel, p_all = binid[o2], rel[o2], p_all[o2]
        cnt = np.bincount(binid, minlength=NBIN)
        counts[c] = cnt
        bounds = np.concatenate([[0], np.cumsum(cnt)])
        per_core_slots.append((rel.astype(np.int32), p_all.astype(np.int32),
                               bounds))

    maxc = counts.max(axis=0).reshape(NT, 2, NWIN)
    nblk = (maxc + 127) // 128                   # blocks per (t, h, w)

    groups = []
    blk_off = 0
    col_off = 0
    for g0 in range(0, NT, GROUP_T):
        tiles = list(range(g0, min(NT, g0 + GROUP_T)))
        calls = []
        for w in range(NWIN):
            wblocks = []                          # [(t, h, blk_within)]
            for t in tiles:
                for h in range(2):
                    for b in range(int(nblk[t, h, w])):
                        wblocks.append((t, h, b))
            i = 0
            while i < len(wblocks):
                chunk = wblocks[i:i + MAX_CALL_BLK]
                calls.append({
                    "w": w,
                    "nb": len(chunk),
                    "col": col_off,
                    "blk": blk_off,
                    "blocks": chunk,             # (tile, half, b_within)
                })
                blk_off += len(chunk)
                col_off += len(chunk) * 8
                i += MAX_CALL_BLK
        groups.append({"tiles": tiles, "calls": calls})
    totblk, totcol = blk_off, col_off

    per_core = []
    for c in range(NCORES):
        rel, part, bounds = per_core_slots[c]
        idx_cols = np.zeros((128, totcol), np.int16)
        pp = np.full((128, totblk), -1.0, np.float32)
        for g in groups:
            for call in g["calls"]:
                w = call["w"]
                stream = np.zeros(call["nb"] * 128, np.int16)
                for j, (t, h, b) in enumerate(call["blocks"]):
                    bid = (t * 2 + h) * NWIN + w
                    lo, hi = bounds[bid], bounds[bid + 1]
                    s0 = lo + b * 128
                    s1 = min(hi, s0 + 128)
                    nsl = s1 - s0
                    if nsl > 0:
                        stream[j * 128:j * 128 + nsl] = rel[s0:s1]
                        pp[:nsl, call["blk"] + j] = part[s0:s1]
                idx_cols[:, call["col"]:call["col"] + call["nb"] * 8] = \
                    _wrap_idx(stream)
        per_core.append({"idx": idx_cols, "pp": pp})

    sched = {"groups": groups, "totblk": totblk, "totcol": totcol}
    return sched, per_core


def _build_program(sched):
    nc = bacc.Bacc("TRN2", target_bir_lowering=False, debug=False,
                   num_devices=NCORES, num_swdge_queues=4)
    totblk, totcol = sched["totblk"], sched["totcol"]

    def inp(name, shape, dt=F32):
        return nc.declare_dram_parameter(name, shape, dt, isOutput=False)

    xT = inp("xT", [CIN, NT * 128])
    dinv = inp("dinv", [128, NT])
    batchf = inp("batchf", [128, NT])
    iota01 = inp("iota01", [128, 256])
    iota128 = inp("iota128", [128, 128])
    ident = inp("ident", [128, 128])
    idxT = inp("idx", [128, totcol], I16)
    ppT = inp("pp", [128, totblk])
    W1 = inp("W1", [CIN, H1]);  Wr1 = inp("Wr1", [CIN, H1])
    W2 = inp("W2", [H1, H2]);   Wr2 = inp("Wr2", [H1, H2])
    b1b = inp("b1b", [128, H1]); br1b = inp("br1b", [128, H1])
    b2b = inp("b2b", [128, H2]); br2b = inp("br2b", [128, H2])
    Wf1t = inp("Wf1t", [16, 80])
    Wf2 = inp("Wf2", [80, NCLS])
    bf2r = inp("bf2r", [1, NCLS])
    mcin = inp("mcin", [1, 80])
    alpha = inp("alpha", [128, 2])
    out = nc.declare_dram_parameter("out", [G, NCLS], F32, isOutput=True)

    SILU = mybir.ActivationFunctionType.Silu
    EQ = mybir.AluOpType.is_equal

    with tile.TileContext(nc) as tc:
        with tc.tile_pool(name="const", bufs=1) as constp, \
             tc.tile_pool(name="store", bufs=1) as storep, \
             tc.tile_pool(name="xp", bufs=4) as xp, \
             tc.tile_pool(name="msg", bufs=24) as msgp, \
             tc.tile_pool(name="stl", bufs=24) as stlp, \
             tc.tile_pool(name="idxp", bufs=6) as idxp, \
             tc.tile_pool(name="ppp", bufs=6) as ppp, \
             tc.tile_pool(name="ep", bufs=6) as ep, \
             tc.tile_pool(name="dram", bufs=1, space="DRAM") as dram, \
             tc.tile_pool(name="ps_tp", bufs=1, space="PSUM") as ps_tp, \
             tc.tile_pool(name="ps_mm", bufs=2, space="PSUM") as ps_mm, \
             tc.tile_pool(name="ps_agg", bufs=4, space="PSUM") as ps_agg, \
             tc.tile_pool(name="ps_pool", bufs=1, space="PSUM") as ps_pool:

            nc.gpsimd.load_library(mlp_lib)

            def ld(ap_src, shape, dt=F32, tag=None):
                t = constp.tile(shape, dt, tag=tag or ap_src.tensor.name)
                nc.sync.dma_start(out=t[:], in_=ap_src)
                return t

            dinv_sb = ld(dinv[:], [128, NT])
            batch_sb = ld(batchf[:], [128, NT])
            iota_sb = ld(iota01[:], [128, 256])
            io128_sb = ld(iota128[:], [128, 128])
            ident_sb = ld(ident[:], [128, 128])
            W1_sb = ld(W1[:], [CIN, H1]); Wr1_sb = ld(Wr1[:], [CIN, H1])
            W2_sb = ld(W2[:], [H1, H2]); Wr2_sb = ld(Wr2[:], [H1, H2])
            b1_sb = ld(b1b[:], [128, H1]); br1_sb = ld(br1b[:], [128, H1])
            b2_sb = ld(b2b[:], [128, H2]); br2_sb = ld(br2b[:], [128, H2])
            Wf1t_sb = ld(Wf1t[:], [16, 80])
            Wf2_sb = ld(Wf2[:], [80, NCLS])
            bf2_sb = ld(bf2r[:], [1, NCLS])
            al_sb = ld(alpha[:], [128, 2])
            ones1 = constp.tile([1, 128], F32, tag="ones1")
            nc.vector.memset(ones1[:], 1.0)

            r1_store = storep.tile([128, NT * H1], F32, tag="r1s")
            r2_store = storep.tile([128, NT * H2], F32, tag="r2s")
            h1sc_store = storep.tile([128, NT * H1], BF, tag="h1sc")
            h2sc_store = storep.tile([128, NT * H1], BF, tag="h2sc")

            h1s_shard = dram.tile([HROWS, 128], BF)
            table1 = dram.tile([TROWS, 128], BF)
            h2s_shard = dram.tile([HROWS, 128], BF)
            table2 = dram.tile([TROWS, 128], BF)
            pool_in = dram.tile([G, 17], F32)
            pool_out = dram.tile([G, 17], F32)

            HB = 49 * H1

            def write_shard(store, shard):
                # shard row r: low half = node r, high half = node 6272+r
                nc.scalar.dma_start(
                    out=shard[:, 0:64].rearrange("(t p) c -> p t c", p=128),
                    in_=store[:, :HB])
                nc.scalar.dma_start(
                    out=shard[:, 64:128].rearrange("(t p) c -> p t c", p=128),
                    in_=store[:, HB:2 * HB])

            def ag(shard, table_):
                nc.gpsimd.collective_compute(
                    "AllGather", mybir.AluOpType.bypass,
                    replica_groups=[list(range(NCORES))],
                    ins=[shard.opt()], outs=[table_.opt()])

            # ---------------- stage 0: h1sc + r1 ----------------
            XB = 4

            def stage0_run(tlist):
                i = 0
                while i < len(tlist):
                    t0 = tlist[i]
                    nt = 1
                    while (nt < XB and i + nt < len(tlist)
                           and tlist[i + nt] == t0 + nt):
                        nt += 1
                    xw = xp.tile([128, XB * 128], F32, tag="xw")
                    nc.scalar.dma_start(
                        out=xw[:, :nt * 128],
                        in_=xT[:, t0 * 128:(t0 + nt) * 128])
                    for a in range(nt):
                        t = t0 + a
                        xTt = xw[:, a * 128:(a + 1) * 128]
                        hpre = ps_mm.tile([128, H1], F32, tag="mm")
                        nc.tensor.matmul(out=hpre[:], lhsT=xTt, rhs=W1_sb[:],
                                         start=True, stop=True)
                        nc.vector.tensor_scalar_mul(
                            out=h1sc_store[:, t * H1:(t + 1) * H1],
                            in0=hpre[:], scalar1=dinv_sb[:, t:t + 1])

                        r1ps = ps_mm.tile([128, H1], F32, tag="mm")
                        nc.tensor.matmul(out=r1ps[:], lhsT=xTt, rhs=Wr1_sb[:],
                                         start=True, stop=True)
                        r1a = ep.tile([128, H1], F32, tag="r1a")
                        nc.vector.tensor_add(out=r1a[:], in0=r1ps[:],
                                             in1=br1_sb[:])
                        nc.scalar.activation(out=r1a[:], in_=r1a[:], func=SILU)
                        nc.vector.tensor_scalar_mul(
                            out=r1_store[:, t * H1:(t + 1) * H1], in0=r1a[:],
                            scalar1=al_sb[:, 0:1])
                    i += nt

            stage0_run(list(range(NT)))
            write_shard(h1sc_store, h1s_shard)
            ag(h1s_shard, table1)

            qctr = [0]

            def run_groups(table_ab, epilogue, hook=None):
                for gi, g in enumerate(sched["groups"]):
                    tiles = g["tiles"]
                    calls = g["calls"]
                    if not calls:
                        for pos, t in enumerate(tiles):
                            epilogue(t, None, pos, len(tiles))
                        continue
                    gcol0 = calls[0]["col"]
                    gblk0 = calls[0]["blk"]
                    gcols = calls[-1]["col"] + calls[-1]["nb"] * 8 - gcol0
                    gblks = calls[-1]["blk"] + calls[-1]["nb"] - gblk0
                    it = idxp.tile([128, gcols], I16, tag="it",
                                   name=f"it{tiles[0]}")
                    nc.scalar.dma_start(out=it[:],
                                        in_=idxT[:, gcol0:gcol0 + gcols])
                    pt = ppp.tile([128, gblks], F32, tag="pt",
                                  name=f"pt{tiles[0]}")
                    nc.scalar.dma_start(out=pt[:],
                                        in_=ppT[:, gblk0:gblk0 + gblks])

                    agg_g = ps_agg.tile([128, GROUP_T * H1], F32, tag="agg",
                                        name=f"agg{tiles[0]}")
                    nc.vector.memset(agg_g[:], 0.0)
                    aggs = {}
                    nb_left = {}
                    for pos, t in enumerate(tiles):
                        aggs[t] = agg_g[:, pos * H1:(pos + 1) * H1]
                        nb_left[t] = sum(
                            1 for call in calls for (tt, _, _) in call["blocks"]
                            if tt == t)

                    for call in calls:
                        nb = call["nb"]
                        mt = msgp.tile([128, MAX_CALL_BLK * 128], BF, tag="mt")
                        st = stlp.tile([128, MAX_CALL_BLK * 128], BF, tag="st")
                        cb0 = call["blk"] - gblk0
                        nc.vector.tensor_tensor(
                            out=st[:, :nb * 128].rearrange(
                                "p (b d) -> p b d", d=128),
                            in0=pt[:, cb0:cb0 + nb].rearrange(
                                "p b -> p b ()").to_broadcast([128, nb, 128]),
                            in1=io128_sb[:].rearrange(
                                "p d -> p () d").to_broadcast([128, nb, 128]),
                            op=EQ)
                        w = call["w"]
                        nc.gpsimd.dma_gather(
                            mt[:, :nb * 128].rearrange("p (b d) -> p b d",
                                                       d=128),
                            table_ab[w],
                            it[:, call["col"] - gcol0:
                               call["col"] - gcol0 + nb * 8],
                            nb * 128, nb * 128, 128,
                            queue_num=qctr[0] % 4,
                        )
                        qctr[0] += 1
                        for j, (t, h, _) in enumerate(call["blocks"]):
                            nb_left[t] -= 1
                            nc.tensor.matmul(
                                out=aggs[t],
                                lhsT=st[:, j * 128:(j + 1) * 128],
                                rhs=mt[:, j * 128 + h * H1:
                                       j * 128 + h * H1 + H1],
                                start=False, stop=(nb_left[t] == 0),
                                skip_group_check=True)
                    for pos, t in enumerate(tiles):
                        epilogue(t, aggs[t], pos, len(tiles))
                    if hook is not None:
                        hook(gi)

            # ---------------- stage 1 ----------------
            def epi1(t, agg, pos, ntl):
                asum = ep.tile([128, H1], F32, tag="e1s")
                nc.vector.tensor_add(out=asum[:], in0=agg,
                                     in1=h1sc_store[:, t * H1:(t + 1) * H1])
                a = ep.tile([128, H1], F32, tag="e1a")
                nc.vector.tensor_scalar_mul(out=a[:], in0=asum[:],
                                            scalar1=dinv_sb[:, t:t + 1])
                nc.vector.tensor_add(out=a[:], in0=a[:], in1=b1_sb[:])
                nc.scalar.activation(out=a[:], in_=a[:], func=SILU)
                h = ep.tile([128, H1], F32, tag="e1h")
                nc.vector.tensor_add(out=h[:], in0=a[:],
                                     in1=r1_store[:, t * H1:(t + 1) * H1])
                nc.vector.tensor_scalar_mul(
                    out=h2sc_store[:, t * H1:(t + 1) * H1], in0=h[:],
                    scalar1=dinv_sb[:, t:t + 1])
                hT_ps = ps_tp.tile([128, 128], F32, tag="tp")
                nc.tensor.transpose(out=hT_ps[:H1, :], in_=h[:],
                                    identity=ident_sb[:])
                hT = ep.tile([H1, 128], F32, tag="e1ht")
                nc.vector.tensor_copy(out=hT[:], in_=hT_ps[:H1, :])
                r2ps = ps_mm.tile([128, H2], F32, tag="mm")
                nc.tensor.matmul(out=r2ps[:], lhsT=hT[:], rhs=Wr2_sb[:],
                                 start=True, stop=True)
                r2a = ep.tile([128, H2], F32, tag="e1r2")
                nc.vector.tensor_add(out=r2a[:], in0=r2ps[:], in1=br2_sb[:])
                nc.scalar.activation(out=r2a[:], in_=r2a[:], func=SILU)
                nc.vector.tensor_scalar_mul(
                    out=r2_store[:, t * H2:(t + 1) * H2], in0=r2a[:],
                    scalar1=al_sb[:, 1:2])

            run_groups((table1[0:WIN, :], table1[WIN:2 * WIN, :]), epi1)

            write_shard(h2sc_store, h2s_shard)
            ag(h2s_shard, table2)

            # ---------------- stage 2 + pooling ----------------
            pool_ps = ps_pool.tile([128, 34], F32, tag="pool")
            nc.vector.memset(pool_ps[:], 0.0)
            tcount = [0]

            def epi2(t, agg, pos, ntl):
                asum = ep.tile([128, H1], F32, tag="e2s")
                nc.vector.tensor_add(out=asum[:], in0=agg,
                                     in1=h2sc_store[:, t * H1:(t + 1) * H1])
                a = ep.tile([128, H1], F32, tag="e2a")
                nc.vector.tensor_scalar_mul(out=a[:], in0=asum[:],
                                            scalar1=dinv_sb[:, t:t + 1])
                aT_ps = ps_tp.tile([128, 128], F32, tag="tp")
                nc.tensor.transpose(out=aT_ps[:H1, :], in_=a[:],
                                    identity=ident_sb[:])
                aT = ep.tile([H1, 128], F32, tag="e2at")
                nc.vector.tensor_copy(out=aT[:], in_=aT_ps[:H1, :])
                zps = ps_mm.tile([128, H2], F32, tag="mm")
                nc.tensor.matmul(out=zps[:], lhsT=aT[:], rhs=W2_sb[:],
                                 start=True, stop=True)
                zext = ep.tile([128, H2 + 1], F32, tag="e2z")
                nc.vector.tensor_add(out=zext[:, :H2], in0=zps[:], in1=b2_sb[:])
                nc.vector.tensor_add(out=zext[:, :H2], in0=zext[:, :H2],
                                     in1=r2_store[:, t * H2:(t + 1) * H2])
                nc.vector.memset(zext[:, H2:], 1.0)
                s0 = ep.tile([128, 128], F32, tag="e2s0")
                nc.vector.tensor_tensor(
                    out=s0[:], in0=batch_sb[:, t:t + 1].to_broadcast([128, 128]),
                    in1=iota_sb[:, 0:128], op=EQ)
                k = tcount[0]
                nc.tensor.matmul(out=pool_ps[:, 0:17], lhsT=s0[:], rhs=zext[:],
                                 start=False, stop=(k == NT - 1),
                                 skip_group_check=True)
                s1 = ep.tile([128, 128], F32, tag="e2s1")
                nc.vector.tensor_tensor(
                    out=s1[:], in0=batch_sb[:, t:t + 1].to_broadcast([128, 128]),
                    in1=iota_sb[:, 128:256], op=EQ)
                nc.tensor.matmul(out=pool_ps[:, 17:34], lhsT=s1[:], rhs=zext[:],
                                 start=False, stop=(k == NT - 1),
                                 skip_group_check=True)
                tcount[0] += 1

            run_groups((table2[0:WIN, :], table2[WIN:2 * WIN, :]), epi2)

            psums = ep.tile([128, 34], F32, tag="psums")
            nc.vector.tensor_copy(out=psums[:], in_=pool_ps[:])
            nc.sync.dma_start(out=pool_in[0:128, :], in_=psums[:, 0:17])
            nc.sync.dma_start(out=pool_in[128:256, :], in_=psums[:, 17:34])

            nc.gpsimd.collective_compute(
                "AllReduce", mybir.AluOpType.add,
                replica_groups=[list(range(NCORES))],
                ins=[pool_in.opt()], outs=[pool_out.opt()])

            mc = ep.tile([1, 80], F32, tag="mmc")
            nc.sync.dma_start(out=mc[:], in_=mcin[:])

            # ---------------- classifier head (two graph windows) ----------
            for wdw in range(2):
                sums = ep.tile([128, 17], F32, tag="hsum")
                nc.sync.dma_start(out=sums[:],
                                  in_=pool_out[wdw * 128:(wdw + 1) * 128, :])
                cnt = ep.tile([128, 1], F32, tag="hcnt")
                nc.vector.tensor_scalar_max(out=cnt[:], in0=sums[:, 16:17],
                                            scalar1=1.0)
                rec = ep.tile([128, 1], F32, tag="hrec")
                nc.vector.reciprocal(out=rec[:], in_=cnt[:])
                ge = ep.tile([128, 16], F32, tag="hge")
                nc.vector.tensor_scalar_mul(out=ge[:], in0=sums[:, :16],
                                            scalar1=rec[:])
                geT_ps = ps_tp.tile([128, 128], F32, tag="tp")
                nc.tensor.transpose(out=geT_ps[:16, :], in_=ge[:],
                                    identity=ident_sb[:])
                geT = ep.tile([16, 128], F32, tag="hget")
                nc.vector.tensor_copy(out=geT[:], in_=geT_ps[:16, :])
                u_ps = ps_mm.tile([128, 80], F32, tag="mm")
                nc.tensor.matmul(out=u_ps[:], lhsT=geT[:], rhs=Wf1t_sb[:],
                                 start=True, stop=False)
                nc.tensor.matmul(out=u_ps[:], lhsT=ones1[:], rhs=mc[:],
                                 start=False, stop=True)
                u = ep.tile([128, 80], F32, tag="hu")
                nc.scalar.activation(out=u[:], in_=u_ps[:], func=SILU)
                uT_ps = ps_tp.tile([128, 128], F32, tag="tp")
                nc.tensor.transpose(out=uT_ps[:80, :], in_=u[:],
                                    identity=ident_sb[:])
                uT = ep.tile([80, 128], F32, tag="hut")
                nc.vector.tensor_copy(out=uT[:], in_=uT_ps[:80, :])
                o_ps = ps_mm.tile([128, NCLS], F32, tag="mm")
                nc.tensor.matmul(out=o_ps[:], lhsT=uT[:], rhs=Wf2_sb[:],
                                 start=True, stop=False)
                nc.tensor.matmul(out=o_ps[:], lhsT=ones1[:], rhs=bf2_sb[:],
                                 start=False, stop=True)
                o = ep.tile([128, NCLS], F32, tag="ho")
                nc.vector.tensor_copy(out=o[:], in_=o_ps[:])
                nc.sync.dma_start(out=out[wdw * 128:(wdw + 1) * 128, :],
                                  in_=o[:])

    nc.compile()
    return nc


def _host_metrics_contrib(tolerance, cost, time, quantity,
                          mW1, mb1, mW2, mb2, Wf1, bf1):
    silu = lambda v: v / (1.0 + np.exp(-v))
    m = np.stack([np.asarray(v, np.float32).reshape(1, 1) for v in
                  (tolerance, cost, time, quantity)])         # [4,1,1]
    e = silu(np.einsum('gij,gjk->gik', m, np.asarray(mW1, np.float32))
             + np.asarray(mb1, np.float32)[:, None, :])
    e = (np.einsum('gij,gjk->gik', e, np.asarray(mW2, np.float32))
         + np.asarray(mb2, np.float32)[:, None, :])           # [4,1,16]
    metvec = e.transpose(1, 0, 2).reshape(1, 64)
    mc = (metvec @ np.asarray(Wf1, np.float32)[16:, :]
          + np.asarray(bf1, np.float32)[None, :])
    return mc.astype(np.float32)


def kernel(x, edge_index, batch, tolerance, cost, time, quantity,
           W1, b1, W2, b2, Wr1, br1, Wr2, br2, alpha1, alpha2,
           mW1, mb1, mW2, mb2, Wf1, bf1, Wf2, bf2):
    x = np.asarray(x, np.float32)
    src = np.asarray(edge_index[0], np.int64).astype(np.int64)
    dst = np.asarray(edge_index[1], np.int64).astype(np.int64)
    batch = np.asarray(batch, np.int64)

    deg = 1.0 + np.bincount(dst, minlength=N).astype(np.float32)
    dinv_full = 1.0 / np.sqrt(deg)

    sched, per_core = _build_structure(src, dst)
    nc = _build_program(sched)

    iota01 = np.tile(np.arange(256, dtype=np.float32), (128, 1))
    iota128 = np.tile(np.arange(128, dtype=np.float32), (128, 1))
    ident = np.eye(128, dtype=np.float32)
    common = {
        "iota01": iota01, "iota128": iota128, "ident": ident,
        "W1": np.asarray(W1, np.float32), "Wr1": np.asarray(Wr1, np.float32),
        "W2": np.asarray(W2, np.float32), "Wr2": np.asarray(Wr2, np.float32),
        "b1b": np.tile(np.asarray(b1, np.float32), (128, 1)),
        "br1b": np.tile(np.asarray(br1, np.float32), (128, 1)),
        "b2b": np.tile(np.asarray(b2, np.float32), (128, 1)),
        "br2b": np.tile(np.asarray(br2, np.float32), (128, 1)),
        "Wf1t": np.asarray(Wf1[:16, :], np.float32),
        "Wf2": np.asarray(Wf2, np.float32),
        "bf2r": np.asarray(bf2, np.float32)[None, :],
        "mcin": _host_metrics_contrib(tolerance, cost, time, quantity,
                                      mW1, mb1, mW2, mb2, Wf1, bf1),
        "alpha": np.tile(np.array([[float(alpha1), float(alpha2)]],
                                  np.float32), (128, 1)),
    }

    in_maps = []
    for c in range(NCORES):
        lo, hi = c * SHARD, (c + 1) * SHARD
        xs = np.zeros((NT * 128, CIN), np.float32)
        xs[:SHARD] = x[lo:hi]
        dv = np.zeros(NT * 128, np.float32)
        dv[:SHARD] = dinv_full[lo:hi]
        bf_loc = np.full(NT * 128, -1.0, np.float32)
        bf_loc[:SHARD] = batch[lo:hi].astype(np.float32)
        m = dict(common)
        m["xT"] = np.ascontiguousarray(xs.T)
        m["dinv"] = dv.reshape(NT, 128).T.copy()
        m["batchf"] = bf_loc.reshape(NT, 128).T.copy()
        m["idx"] = per_core[c]["idx"]
        m["pp"] = per_core[c]["pp"]
        in_maps.append(m)

    res = run_bass_kernel_spmd(nc, in_maps, list(range(NCORES)))
    kernel._last = (nc, in_maps)   # for external profiling harnesses
    kernel._res = res
    return np.asarray(res.results[0]["out"], np.float32)
